# revision 15
# baseline (speedup 1.0000x reference)
"""FLAME layer on 8 Trainium2 NeuronCores (Bass/Tile).

Strategy (vertex-parallel):
  * V=5023 padded to 5120 and split 640 vertices/core; every core handles
    the full batch B=2048 for its vertex slice. This replicates only the
    small per-batch operands (betas, pose features, joint transforms)
    while the big model tensor (shapedirs, 24MB) is sliced 1/8 per core.
  * Host (O(B) + O(model) prep): rodrigues, forward-kinematic chain,
    A_rel; J is linear in betas (J = J0 + betas @ SJ with SJ precomputed
    from J_regressor x shapedirs), so no device dependency on v_shaped.
    Host also pre-transposes operands so the device does zero transposes.
  * Device (all O(B*V) work), per 128-vertex chunk in [v(part), b(free)]
    layout:
      1. vposed_c = sdt_aug_c.T @ betas_aug  (PE; K=437 = 400 blendshape
         rows + v_template x ones row + 36 posedirs rows; computed as
         bf16 hi/lo 3-pass for ~1e-6 accuracy at bf16 speed)
      2. T = blend of per-joint transforms (PE): rotation planes as one
         K=45 float32r matmul per (plane, b-slice); translation planes
         (which enter verts unattenuated) as K=15 bf16 hi/lo 3-pass
      3. verts_c = T_c0*vp_x + T_c1*vp_y + T_c2*vp_z + T_c3  (DVE)
      4. DMA out [3, 640, 2048]; host reassembles [B, V, 3].
"""

import os
from contextlib import ExitStack

import ml_dtypes
import numpy as np

import bass_rust
import concourse.bass as bass
import concourse.mybir as mybir
import concourse.tile as tile_mod
from concourse.bass_utils import run_bass_kernel_spmd

# ---------------------------------------------------------------- constants
B = 2048
V = 5023
VP = 5120            # padded vertex count (8 cores x 640)
NVC = VP // 8        # vertices per core
NCHUNK = NVC // 128  # 128-vertex chunks per core (5)
NJ = 5
NCORES = 8
BH = 1024            # batch half (free-dim tile for T/apply)
KB = 437             # vposed contraction rows: 400 betas + 1 ones + 36 pose
KCH = [(0, 128), (128, 128), (256, 128), (384, 53)]
NS = 512             # matmul free-dim slice (one PSUM bank)
PARENTS = np.array([-1, 0, 1, 1, 1])

BF16 = ml_dtypes.bfloat16

# ------------------------------------------------- walrus multi-wait patch
# This walrus build accepts only ONE sem-wait per instruction (CTRL and
# LW queue structs alike), but Tile freely assigns several. Split the
# surplus waits onto same-engine NOPs emitted immediately before the
# instruction — the engine stalls on each NOP's wait first, so the
# gating semantics are identical.


def _patched_commit_instruction(self, inst, lazy_reg_writes=True):
    si = inst.sync_info
    if si is not None and len(si.on_wait) > 1:
        waits = list(si.on_wait)
        inst.sync_info = bass_rust.SyncInfo(
            on_update=list(si.on_update), on_wait=waits[:1]
        )
        for w in waits[1:]:
            nop = mybir.InstNoOp(
                name=self.nc.get_next_instruction_name(),
                engine=inst.engine,
                ins=[],
                outs=[],
                bass_nofuse=True,
                sync_info=bass_rust.SyncInfo(on_update=[], on_wait=[w]),
            )
            _orig_commit_instruction(self, nop, lazy_reg_writes=False)
    return _orig_commit_instruction(self, inst, lazy_reg_writes)


def _split_inst_waits(nc, inst):
    si = inst.ins.sync_info
    if si is None:
        return
    waits = list(si.on_wait)
    if len(waits) <= 1:
        return
    inst.ins.sync_info = bass_rust.SyncInfo(
        on_update=list(si.on_update), on_wait=waits[:1]
    )
    for i in range(1, len(waits)):
        nop = nc.sync.nop(nofuse=True, hint="drain_wait_split")
        nop.ins.sync_info = bass_rust.SyncInfo(on_update=[], on_wait=[waits[i]])


def _patched_drain_and_barrier(self, tick_clock, wait_clock):
    nc = self.nc
    drain_inst = nc.sync.drain()
    wait_clock.add_sem_waits(
        drain_inst.ins, tile_mod.ScopedClock({None: tick_clock.global_clock})
    )
    _split_inst_waits(nc, drain_inst)
    nc.all_engine_barrier()
    assert self.sems is not None
    popped = nc._tile_sem_poison_stack.pop()
    assert popped is self._sem_poison
    nc.clear_and_free_semaphores(list(self.sems.allocated().values()))
    nc.all_engine_barrier()


_orig_commit_instruction = tile_mod.TileContext._commit_instruction
if getattr(tile_mod.TileContext, "_flame_wait_patch", False) is False:
    tile_mod.TileContext._commit_instruction = _patched_commit_instruction
    tile_mod.TileContext._drain_and_barrier = _patched_drain_and_barrier
    tile_mod.TileContext._flame_wait_patch = True

# ----------------------------------------------------------- host-side math


def _rodrigues(r):
    angle = np.linalg.norm(r, axis=-1, keepdims=True) + 1e-8
    axis = r / angle
    x, y, z = axis[..., 0], axis[..., 1], axis[..., 2]
    zero = np.zeros_like(x)
    K = np.stack([zero, -z, y, z, zero, -x, -y, x, zero], axis=-1)
    K = K.reshape(r.shape[:-1] + (3, 3))
    s = np.sin(angle)[..., None]
    c = np.cos(angle)[..., None]
    return np.eye(3, dtype=r.dtype) + s * K + (1.0 - c) * (K @ K)


def _split_hi_lo(x):
    hi = x.astype(np.float32).astype(BF16)
    lo = (x.astype(np.float32) - hi.astype(np.float32)).astype(BF16)
    return np.ascontiguousarray(hi), np.ascontiguousarray(lo)


def _host_batch_prep(shape, expression, rotation, neck, jaw, eyeballs,
                     v_template, shapedirs, J_regressor):
    f64 = np.float64
    b = shape.shape[0]
    betas = np.concatenate([shape, expression], axis=1).astype(f64)
    full_pose = np.concatenate([rotation, neck, jaw, eyeballs], axis=1).astype(f64)

    jr = J_regressor.astype(f64)
    J0 = jr @ v_template.astype(f64)                                   # [5,3]
    SJ = np.einsum('jv,vcl->ljc', jr, shapedirs.astype(f64)).reshape(400, NJ * 3)
    J = (J0.reshape(-1) + betas @ SJ).reshape(b, NJ, 3)

    rot_mats = _rodrigues(full_pose.reshape(b, NJ, 3))
    pose_feature = (rot_mats[:, 1:] - np.eye(3, dtype=f64)).reshape(b, 36)

    rel_joints = np.concatenate([J[:, :1], J[:, 1:] - J[:, PARENTS[1:]]], axis=1)
    T_local = np.zeros((b, NJ, 4, 4), dtype=f64)
    T_local[:, :, :3, :3] = rot_mats
    T_local[:, :, :3, 3] = rel_joints
    T_local[:, :, 3, 3] = 1.0
    chain = [T_local[:, 0]]
    for j in range(1, NJ):
        chain.append(chain[PARENTS[j]] @ T_local[:, j])
    A = np.stack(chain, axis=1)

    j_hom = np.concatenate([J, np.zeros_like(J[..., :1])], axis=-1)
    t_corr = np.einsum('bjmn,bjn->bjm', A, j_hom)
    A_rel = A.copy()
    A_rel[:, :, :, 3] -= t_corr                                        # [B,5,4,4]

    # betas_aug [437, B]: rows 0-399 betas.T, 400 ones, 401-436 pose_feature.T
    betas_aug = np.empty((KB, b), dtype=np.float32)
    betas_aug[:400] = betas.T
    betas_aug[400] = 1.0
    betas_aug[401:] = pose_feature.T
    bt_hi, bt_lo = _split_hi_lo(betas_aug)

    # rotation part of A_rel: [45, B], rows j*9 + c*3 + n (n<3) — float32
    arel45 = np.ascontiguousarray(
        A_rel[:, :, :3, :3].transpose(1, 2, 3, 0).reshape(45, b)
    ).astype(np.float32)
    # translation part: [15, B], rows j*3 + c
    arel15 = np.ascontiguousarray(
        A_rel[:, :, :3, 3].transpose(1, 2, 0).reshape(15, b)
    ).astype(np.float32)
    a15_hi, a15_lo = _split_hi_lo(arel15)
    return bt_hi, bt_lo, arel45, a15_hi, a15_lo


def _host_model_prep(v_template, shapedirs, posedirs, lbs_weights):
    # sdt_aug [3, 437, VP] matching betas_aug rows
    sdt = np.zeros((3, KB, VP), dtype=np.float32)
    sdt[:, :400, :V] = shapedirs.transpose(1, 2, 0)
    sdt[:, 400, :V] = v_template.T
    sdt[:, 401:, :V] = posedirs.reshape(36, V, 3).transpose(2, 0, 1)
    sdt_hi, sdt_lo = _split_hi_lo(sdt)

    # w45 [45, 9, VP]: for rotation plane p=(c,n), row j*9+c*3+n = w[v,j]
    w45 = np.zeros((45, 9, VP), dtype=np.float32)
    for c in range(3):
        for n in range(3):
            p = c * 3 + n
            for j in range(NJ):
                w45[j * 9 + p, p, :V] = lbs_weights[:, j]
    # w15 [15, 3, VP]: for translation plane c, row j*3+c = w[v,j]
    w15 = np.zeros((15, 3, VP), dtype=np.float32)
    for c in range(3):
        for j in range(NJ):
            w15[j * 3 + c, c, :V] = lbs_weights[:, j]
    w15_hi, w15_lo = _split_hi_lo(w15)
    return sdt_hi, sdt_lo, w45, w15_hi, w15_lo

# ------------------------------------------------------------ device kernel


def _build_device_program():
    nc = bass.Bass("TRN2", target_bir_lowering=False, debug=False)
    f32 = mybir.dt.float32
    f32r = mybir.dt.float32r
    bf16 = mybir.dt.bfloat16

    sdt_hi = nc.dram_tensor("sdt_hi", [3, KB, NVC], bf16, kind="ExternalInput").ap()
    sdt_lo = nc.dram_tensor("sdt_lo", [3, KB, NVC], bf16, kind="ExternalInput").ap()
    w45 = nc.dram_tensor("w45", [45, 9 * NVC], f32r, kind="ExternalInput").ap()
    w15_hi = nc.dram_tensor("w15_hi", [15, 3 * NVC], bf16, kind="ExternalInput").ap()
    w15_lo = nc.dram_tensor("w15_lo", [15, 3 * NVC], bf16, kind="ExternalInput").ap()
    bt_hi = nc.dram_tensor("bt_hi", [KB, B], bf16, kind="ExternalInput").ap()
    bt_lo = nc.dram_tensor("bt_lo", [KB, B], bf16, kind="ExternalInput").ap()
    arel45 = nc.dram_tensor("arel45", [45, B], f32r, kind="ExternalInput").ap()
    a15_hi = nc.dram_tensor("a15_hi", [15, B], bf16, kind="ExternalInput").ap()
    a15_lo = nc.dram_tensor("a15_lo", [15, B], bf16, kind="ExternalInput").ap()
    out = nc.dram_tensor("out", [3, NVC, B], f32, kind="ExternalOutput").ap()

    with tile_mod.TileContext(nc) as tc, ExitStack() as ctx:
        cpool = ctx.enter_context(tc.tile_pool(name="const", bufs=1))
        spool = ctx.enter_context(tc.tile_pool(name="stream", bufs=2))
        vpool = ctx.enter_context(tc.tile_pool(name="vposed", bufs=1))
        tpool = ctx.enter_context(tc.tile_pool(name="tblend", bufs=1))
        apool = ctx.enter_context(tc.tile_pool(name="apply", bufs=2))
        ps_v = ctx.enter_context(tc.tile_pool(name="psv", bufs=4, space="PSUM"))
        ps_t = ctx.enter_context(tc.tile_pool(name="pst", bufs=4, space="PSUM"))

        # resident operands
        bth, btl = [], []
        for ki, (k0, kn) in enumerate(KCH):
            th = cpool.tile([kn, B], bf16, tag=f"bth{ki}")
            nc.sync.dma_start(th[:], bt_hi[k0:k0 + kn, :])
            bth.append(th)
            tl = cpool.tile([kn, B], bf16, tag=f"btl{ki}")
            nc.sync.dma_start(tl[:], bt_lo[k0:k0 + kn, :])
            btl.append(tl)
        w45t = cpool.tile([45, 9 * NVC], f32r, tag="w45t")
        nc.sync.dma_start(w45t[:], w45[:, :])
        w15ht = cpool.tile([15, 3 * NVC], bf16, tag="w15ht")
        nc.sync.dma_start(w15ht[:], w15_hi[:, :])
        w15lt = cpool.tile([15, 3 * NVC], bf16, tag="w15lt")
        nc.sync.dma_start(w15lt[:], w15_lo[:, :])
        a45t = cpool.tile([45, B], f32r, tag="a45t")
        nc.sync.dma_start(a45t[:], arel45[:, :])
        a15ht = cpool.tile([15, B], bf16, tag="a15ht")
        nc.sync.dma_start(a15ht[:], a15_hi[:, :])
        a15lt = cpool.tile([15, B], bf16, tag="a15lt")
        nc.sync.dma_start(a15lt[:], a15_lo[:, :])

        for k in range(NCHUNK):
            vs = slice(k * 128, (k + 1) * 128)

            # stream this chunk's vposed lhsT tiles (hi/lo bf16)
            sh, sl = [], []
            for c in range(3):
                rh, rl = [], []
                for ki, (k0, kn) in enumerate(KCH):
                    t = spool.tile([kn, 128], bf16, tag=f"sh{c}_{ki}")
                    nc.sync.dma_start(t[:], sdt_hi[c, k0:k0 + kn, vs])
                    rh.append(t)
                    t = spool.tile([kn, 128], bf16, tag=f"sl{c}_{ki}")
                    nc.sync.dma_start(t[:], sdt_lo[c, k0:k0 + kn, vs])
                    rl.append(t)
                sh.append(rh)
                sl.append(rl)

            # 1) vposed planes [128, B]: hi*hi + hi*lo + lo*hi, K=437
            vp = []
            for c in range(3):
                dst = vpool.tile([128, B], mybir.dt.float32, tag=f"vp{c}")
                for ns in range(B // NS):
                    bs = slice(ns * NS, (ns + 1) * NS)
                    acc = ps_v.tile([128, NS], mybir.dt.float32, tag="psv")
                    passes = [(sh[c], bth), (sh[c], btl), (sl[c], bth)]
                    for pi, (lhs_row, rhs_row) in enumerate(passes):
                        for ki in range(4):
                            nc.tensor.matmul(
                                acc[:], lhsT=lhs_row[ki][:],
                                rhs=rhs_row[ki][:, bs],
                                start=(pi == 0 and ki == 0),
                                stop=(pi == 2 and ki == 3))
                    nc.scalar.copy(out=dst[:, bs], in_=acc[:])
                vp.append(dst)

            # 2+3) per batch-half: T blend then affine apply
            for h in range(B // BH):
                hb = slice(h * BH, (h + 1) * BH)
                tt = tpool.tile([128, 12 * BH], mybir.dt.float32, tag="tt")
                # rotation planes: K=45 float32r, plane p=(c,n) -> cn=4c+n
                for c in range(3):
                    for n in range(3):
                        p = c * 3 + n
                        cn = 4 * c + n
                        wsl = w45t[:, p * NVC + k * 128:p * NVC + (k + 1) * 128]
                        for ns in range(BH // NS):
                            fs = slice(cn * BH + ns * NS, cn * BH + (ns + 1) * NS)
                            src = slice(h * BH + ns * NS, h * BH + (ns + 1) * NS)
                            acc = ps_t.tile([128, NS], mybir.dt.float32, tag="pst")
                            nc.tensor.matmul(acc[:], lhsT=wsl, rhs=a45t[:, src],
                                             start=True, stop=True)
                            nc.scalar.copy(out=tt[:, fs], in_=acc[:])
                # translation planes: K=15 bf16 hi/lo 3-pass, cn=4c+3
                for c in range(3):
                    cn = 4 * c + 3
                    whsl = w15ht[:, c * NVC + k * 128:c * NVC + (k + 1) * 128]
                    wlsl = w15lt[:, c * NVC + k * 128:c * NVC + (k + 1) * 128]
                    for ns in range(BH // NS):
                        fs = slice(cn * BH + ns * NS, cn * BH + (ns + 1) * NS)
                        src = slice(h * BH + ns * NS, h * BH + (ns + 1) * NS)
                        acc = ps_t.tile([128, NS], mybir.dt.float32, tag="pst")
                        nc.tensor.matmul(acc[:], lhsT=whsl, rhs=a15ht[:, src],
                                         start=True, stop=False)
                        nc.tensor.matmul(acc[:], lhsT=whsl, rhs=a15lt[:, src],
                                         start=False, stop=False)
                        nc.tensor.matmul(acc[:], lhsT=wlsl, rhs=a15ht[:, src],
                                         start=False, stop=True)
                        nc.scalar.copy(out=tt[:, fs], in_=acc[:])

                for c in range(3):
                    def tsl(n):
                        return tt[:, (4 * c + n) * BH:(4 * c + n + 1) * BH]
                    ma = apool.tile([128, BH], mybir.dt.float32, tag="ma")
                    mb = apool.tile([128, BH], mybir.dt.float32, tag="mb")
                    nc.vector.tensor_mul(ma[:], tsl(0), vp[0][:, hb])
                    nc.vector.tensor_mul(mb[:], tsl(1), vp[1][:, hb])
                    nc.vector.tensor_add(ma[:], ma[:], mb[:])
                    nc.vector.tensor_mul(mb[:], tsl(2), vp[2][:, hb])
                    nc.vector.tensor_add(mb[:], mb[:], tsl(3))
                    nc.vector.tensor_add(ma[:], ma[:], mb[:])
                    nc.sync.dma_start(out[c, vs, hb], ma[:])
    return nc


_NC_CACHE = {}


def _get_nc():
    if "nc" not in _NC_CACHE:
        _NC_CACHE["nc"] = _build_device_program()
    return _NC_CACHE["nc"]

# ---------------------------------------------------------------- entry


def build_in_maps(shape, expression, rotation, neck, jaw, eyeballs,
                  v_template, shapedirs, posedirs, J_regressor, lbs_weights):
    bt_hi, bt_lo, arel45, a15_hi, a15_lo = _host_batch_prep(
        shape, expression, rotation, neck, jaw, eyeballs,
        v_template, shapedirs, J_regressor)
    sdt_hi, sdt_lo, w45, w15_hi, w15_lo = _host_model_prep(
        v_template, shapedirs, posedirs, lbs_weights)

    in_maps = []
    for i in range(NCORES):
        v0, v1 = i * NVC, (i + 1) * NVC
        in_maps.append({
            "sdt_hi": np.ascontiguousarray(sdt_hi[:, :, v0:v1]),
            "sdt_lo": np.ascontiguousarray(sdt_lo[:, :, v0:v1]),
            "w45": np.ascontiguousarray(
                w45[:, :, v0:v1]).reshape(45, 9 * NVC),
            "w15_hi": np.ascontiguousarray(
                w15_hi[:, :, v0:v1]).reshape(15, 3 * NVC),
            "w15_lo": np.ascontiguousarray(
                w15_lo[:, :, v0:v1]).reshape(15, 3 * NVC),
            "bt_hi": bt_hi,
            "bt_lo": bt_lo,
            "arel45": arel45,
            "a15_hi": a15_hi,
            "a15_lo": a15_lo,
        })
    return in_maps


def kernel(shape, expression, rotation, neck, jaw, eyeballs,
           v_template, shapedirs, posedirs, J_regressor, lbs_weights):
    in_maps = build_in_maps(shape, expression, rotation, neck, jaw, eyeballs,
                            v_template, shapedirs, posedirs, J_regressor,
                            lbs_weights)
    nc = _get_nc()
    res = run_bass_kernel_spmd(nc, in_maps, core_ids=list(range(NCORES)))

    full = np.concatenate([res.results[i]["out"] for i in range(NCORES)], axis=1)
    verts = np.ascontiguousarray(full[:, :V, :].transpose(2, 1, 0))
    return verts.astype(np.float32)
